# revision 28
# baseline (speedup 1.0000x reference)
"""Trainium2 Bass kernel for nn_EvidentialLoNet (8-core data parallel).

Per core (B_local=512): CNN backbone (3 convs via f32r matmuls with
oh-pair column packing / ky-parity K-packing), 2-layer GRU (feature-major,
per-gate psums with matmul-accumulated input projections), evidential
heads + fusion. Everything on device except batch sharding and weight
packing.
"""
import numpy as np

import concourse.bacc as bacc
import concourse.mybir as mybir
from concourse import tile
from concourse.bass_utils import run_bass_kernel_spmd

AF = mybir.ActivationFunctionType
ALU = mybir.AluOpType
DT = mybir.dt
F32 = DT.float32
F32R = DT.float32r

N_CORES = 8
BL = 512          # batch per core
BCH = 32          # conv chunk batch
NCH = BL // BCH
EPS = 1e-5


# ------------------------------------------------------------------ packing

def _bn_fold(bp):
    s = np.asarray(bp['g'], np.float32) / np.sqrt(np.asarray(bp['v'], np.float32) + EPS)
    t = np.asarray(bp['b'], np.float32) - np.asarray(bp['m'], np.float32) * s
    return s, t


def build_packs(p):
    P = {k: np.asarray(v, np.float32) for k, v in p.items() if not isinstance(v, dict)}
    bn = {k: p[k] for k in ('bn1', 'bn2', 'bn3', 'bn_gru', 'hg_bn', 'hc_bn')}
    o = {}

    s1, t1 = _bn_fold(bn['bn1'])
    w1 = P['conv1_w'] * s1[:, None, None, None]
    b1 = P['conv1_b'] * s1 + t1
    s2, t2 = _bn_fold(bn['bn2'])
    w2 = P['conv2_w'] * s2[:, None, None, None]
    b2 = P['conv2_b'] * s2 + t2
    s3, t3 = _bn_fold(bn['bn3'])
    w3 = P['conv3_w'] * s3[:, None, None, None]
    b3 = P['conv3_b'] * s3 + t3

    w1p = np.zeros((15, 72, 128), np.float32)
    for kx in range(15):
        for ky in range(15):
            for ic in range(4):
                w1p[kx, ky * 4 + ic, 0:64] = w1[:, ic, ky, kx]
                w1p[kx, (ky + 3) * 4 + ic, 64:128] = w1[:, ic, ky, kx]
    o['w1p'] = w1p
    o['b1v'] = np.concatenate([b1, b1]).reshape(128, 1)

    w2p = np.zeros((15, 128, 128), np.float32)
    for kyg in range(3):
        for kx in range(5):
            for d in range(2):
                ky = 2 * kyg + d
                if ky < 5:
                    w2p[kyg * 5 + kx, d * 64:(d + 1) * 64, :] = w2[:, :, ky, kx].T
    o['w2p'] = w2p
    o['b2v'] = b2.reshape(128, 1)

    w3p = np.zeros((2, 25, 128, 128), np.float32)
    for h in range(2):
        for ky in range(5):
            for kx in range(5):
                w3p[h, ky * 5 + kx] = w3[h * 128:(h + 1) * 128, :, ky, kx].T
    o['w3p'] = w3p.reshape(50, 128, 128)
    o['b3a'] = b3[0:128].reshape(128, 1)
    o['b3b'] = b3[128:256].reshape(128, 1)

    fc1w, fc1b = P['fc1_w'], P['fc1_b']
    fc1p = np.zeros((2, 9, 4, 128, 128), np.float32)
    for h in range(2):
        for p2 in range(9):
            for mt in range(4):
                fc1p[h, p2, mt] = fc1w[mt * 128:(mt + 1) * 128,
                                       (h * 128 + np.arange(128)) * 9 + p2].T
    o['fc1p'] = fc1p.reshape(72, 128, 128)
    o['fc1b'] = fc1b.reshape(4, 128).T.copy()       # [128, 4] col per mtile

    W23 = P['fc3_w'] @ P['fc2_w']
    b23 = P['fc3_w'] @ P['fc2_b'] + P['fc3_b']
    w23p = np.zeros((128, 4, 64), np.float32)
    for kc in range(4):
        w23p[:, kc, :] = W23[:, kc * 128:(kc + 1) * 128].T
    o['w23p'] = w23p.reshape(128, 256)
    o['b23v'] = b23.reshape(64, 1)

    for nm, pre in (('hg', 'g'), ('hc', 'c')):
        s, t = _bn_fold(bn[nm + '_bn'])
        W1, B1 = P[nm + '_w1'], P[nm + '_b1']
        W2, B2 = P[nm + '_w2'], P[nm + '_b2']
        o[f'w{pre}1'] = W1.T.copy()
        o[f'b{pre}1'] = B1.reshape(32, 1)
        o[f'w{pre}2'] = (W2 * s[None, :]).T.copy()
        o[f'b{pre}2'] = (W2 @ t + B2).reshape(4, 1)

    for l in range(2):
        wih, whh = P[f'gru{l}_wih'], P[f'gru{l}_whh']
        bih, bhh = P[f'gru{l}_bih'], P[f'gru{l}_bhh']
        o[f'wih{l}T'] = wih.T.copy()                 # [I,48] cols r|z|n
        o[f'whh{l}T'] = whh.T.copy()                 # [16,48]
        o[f'br{l}'] = (bih[0:16] + bhh[0:16]).reshape(16, 1)
        o[f'bz{l}'] = (bih[16:32] + bhh[16:32]).reshape(16, 1)
        o[f'btn{l}'] = bih[32:48].reshape(16, 1)
        o[f'bhh{l}n'] = bhh[32:48].reshape(16, 1)
    sg, tg = _bn_fold(bn['bn_gru'])
    o['clsT'] = (P['cls_w'] * sg[None, :]).T.copy()  # [16,64]
    o['clsb'] = (P['cls_w'] @ tg + P['cls_b']).reshape(64, 1)

    o['ident'] = np.eye(128, dtype=np.float32)
    o['identf'] = np.eye(128, dtype=np.float32)
    o['ones41'] = np.ones((4, 1), np.float32)
    o['bc14'] = np.ones((1, 4), np.float32)
    return o


PACK_SHAPES = {
    'w1p': (15, 72, 128), 'b1v': (128, 1), 'w2p': (15, 128, 128), 'b2v': (128, 1),
    'w3p': (50, 128, 128), 'b3a': (128, 1), 'b3b': (128, 1),
    'fc1p': (72, 128, 128), 'fc1b': (128, 4), 'w23p': (128, 256), 'b23v': (64, 1),
    'wg1': (64, 32), 'bg1': (32, 1), 'wg2': (32, 4), 'bg2': (4, 1),
    'wc1': (64, 32), 'bc1': (32, 1), 'wc2': (32, 4), 'bc2': (4, 1),
    'wih0T': (4, 48), 'whh0T': (16, 48), 'br0': (16, 1), 'bz0': (16, 1),
    'btn0': (16, 1), 'bhh0n': (16, 1),
    'wih1T': (16, 48), 'whh1T': (16, 48), 'br1': (16, 1), 'bz1': (16, 1),
    'btn1': (16, 1), 'bhh1n': (16, 1),
    'clsT': (16, 64), 'clsb': (64, 1),
    'ident': (128, 128), 'identf': (128, 128), 'ones41': (4, 1), 'bc14': (1, 4),
}


# ------------------------------------------------------------------ builder

def build_nc(dbg=False):
    nc = bacc.Bacc("TRN2", target_bir_lowering=False, debug=False, num_devices=1)

    y_d = nc.dram_tensor("y", [BL, 4, 45, 45], F32R, kind="ExternalInput")
    x_d = nc.dram_tensor("x", [BL, 14, 4], F32, kind="ExternalInput")
    R_PACKS = {'w1p', 'w2p', 'w3p', 'w23p', 'wg1', 'wg2', 'wc1', 'wc2',
               'wih0T', 'wih1T', 'whh0T', 'whh1T', 'clsT', 'ones41', 'bc14',
               'ident'}

    def _pdt(n):
        if n == 'fc1p':
            return DT.bfloat16
        return F32R if n in R_PACKS else F32

    pk = {n: nc.dram_tensor(n, list(s), _pdt(n), kind="ExternalInput")
          for n, s in PACK_SHAPES.items()}
    ev_d = nc.dram_tensor("ev_t", [4, BL], F32, kind="ExternalOutput")
    if dbg:
        dbg_xfm = nc.dram_tensor("dbg_xfm", [4, 14, BL], DT.bfloat16, kind="ExternalOutput")
        dbg_h = nc.dram_tensor("dbg_h", [16, 15, BL], F32, kind="ExternalOutput")
        dbg_gf = nc.dram_tensor("dbg_gf", [64, BL], F32, kind="ExternalOutput")
        dbg_xq = nc.dram_tensor("dbg_xq", [56, 4, 128], DT.bfloat16, kind="ExternalOutput")
    a_d = nc.dram_tensor("a_t", [4, 2, BL], F32, kind="ExternalOutput")

    with tile.TileContext(nc) as tc:
        with (
            tc.tile_pool(name="wp", bufs=1) as wp,
            tc.tile_pool(name="main", bufs=1) as mp,
            tc.tile_pool(name="grut", bufs=2) as gp,
            tc.tile_pool(name="gpsum", bufs=1, space="PSUM") as gps,
        ):
            # ---------------- weights into SBUF ----------------
            w1sb = wp.tile([72, 15, 128], F32R, tag="w1")
            nc.sync.dma_start(w1sb[:], pk['w1p'].ap().rearrange("k p m -> p k m"))
            w2sb = wp.tile([128, 15, 128], F32R, tag="w2")
            nc.sync.dma_start(w2sb[:], pk['w2p'].ap().rearrange("k p m -> p k m"))
            w3sb = wp.tile([128, 50, 128], F32R, tag="w3")
            nc.sync.dma_start(w3sb[:], pk['w3p'].ap().rearrange("k p m -> p k m"))

            def _small(name):
                t = wp.tile(list(PACK_SHAPES[name]), _pdt(name), tag=name)
                nc.sync.dma_start(t[:], pk[name].ap())
                return t

            S = {n: _small(n) for n in PACK_SHAPES
                 if n not in ('w1p', 'w2p', 'w3p', 'fc1p')}
            ident = S['ident']

            # persistent activations
            fcin0 = mp.tile([128, BL, 9], DT.bfloat16, tag="fcin0")
            fcin1 = mp.tile([128, BL, 9], DT.bfloat16, tag="fcin1")
            e_sb = mp.tile([4, 2, BL], F32R, tag="e")
            gfeat = mp.tile([64, BL], F32R, tag="gfeat")
            hh0 = mp.tile([16, 15, BL], F32R, tag="hh0")    # L0 h history

            # ---------------- GRU prologue: x -> x_fm ----------------
            with (
                tc.tile_pool(name="prol", bufs=1) as pp,
                tc.tile_pool(name="tpp", bufs=2, space="PSUM") as tpp,
            ):
                xbm = pp.tile([128, 4, 56], F32, tag="xbm")
                nc.sync.dma_start(
                    xbm[:], x_d.ap().rearrange("(c p) t i -> p c (t i)", p=128))
                xq = pp.tile([56, 4, 128], DT.bfloat16, tag="xq")
                for bt in range(4):
                    tps = tpp.tile([56, 128], F32, tag="tp")
                    nc.tensor.transpose(tps[:], xbm[:, bt, :], S['identf'][:])
                    nc.vector.tensor_copy(xq[:, bt, :], tps[:])
                if dbg:
                    nc.sync.dma_start(dbg_xq.ap(), xq[:])
                x_fm = mp.tile([4, 14, BL], DT.bfloat16, tag="x_fm")
                for i in range(4):
                    nc.sync.dma_start(
                        x_fm[i:i + 1, :, :].rearrange("o t (c b) -> o t c b", c=4),
                        xq[i:i + 1 + 4 * 13:4, :, :])

            # ---------------- GRU ----------------
            wih0bf = mp.tile([4, 48], DT.bfloat16, tag="wih0bf")
            nc.vector.tensor_copy(wih0bf[:], S['wih0T'][:])
            nc.gpsimd.memset(hh0[:, 0, :].bitcast(F32), 0.0)
            h1_0 = gp.tile([16, BL], F32R, tag="h1", bufs=2)
            nc.gpsimd.memset(h1_0[:].bitcast(F32), 0.0)

            def gru_step(l, x_ap, h_prev, h_out):
                whh = S[f'whh{l}T']
                if l == 0:
                    xw = lambda a, b: wih0bf[:, a:b]
                    xr = x_ap
                else:
                    xw = lambda a, b: S['wih1T'][:, a:b]
                    xr = x_ap
                psr = gps.tile([16, BL], F32, tag="g16", bufs=4, name="psr")
                nc.tensor.matmul(psr[:], whh[:, 0:16], h_prev,
                                 start=True, stop=False)
                nc.tensor.matmul(psr[:], xw(0, 16), xr,
                                 start=False, stop=True)
                psz = gps.tile([16, BL], F32, tag="g16", bufs=4, name="psz")
                nc.tensor.matmul(psz[:], whh[:, 16:32], h_prev,
                                 start=True, stop=False)
                nc.tensor.matmul(psz[:], xw(16, 32), xr,
                                 start=False, stop=True)
                psn = gps.tile([16, BL], F32, tag="g16", bufs=4, name="psn")
                nc.tensor.matmul(psn[:], whh[:, 32:48], h_prev)
                r = gp.tile([16, BL], F32, tag="r")
                nc.scalar.activation(r[:], psr[:], AF.Sigmoid, bias=S[f'br{l}'][:])
                z = gp.tile([16, BL], F32, tag="z")
                nc.scalar.activation(z[:], psz[:], AF.Sigmoid, bias=S[f'bz{l}'][:])
                nm = gp.tile([16, BL], F32R, tag="nm", bufs=1)
                nc.vector.scalar_tensor_tensor(nm[:], psn[:], S[f'bhh{l}n'][:], r[:],
                                               op0=ALU.add, op1=ALU.mult)
                psna = gps.tile([16, BL], F32, tag="g16", bufs=4, name="psna")
                nc.tensor.matmul(psna[:], xw(32, 48), xr,
                                 start=True, stop=False)
                nc.tensor.matmul(psna[:], ident[0:16, 0:16], nm[:],
                                 start=False, stop=True)
                nt = gp.tile([16, BL], F32, tag="nt")
                nc.scalar.activation(nt[:], psna[:], AF.Tanh, bias=S[f'btn{l}'][:])
                d = gp.tile([16, BL], F32, tag="d", bufs=1)
                nc.vector.tensor_sub(d[:], h_prev, nt[:])
                zd = gp.tile([16, BL], F32, tag="zd", bufs=1)
                nc.vector.tensor_mul(zd[:], z[:], d[:])
                nc.vector.tensor_add(h_out, nt[:], zd[:])

            for t in range(14):
                gru_step(0, x_fm[:, t, :], hh0[:, t, :], hh0[:, t + 1, :])
            h1 = h1_0
            for t in range(14):
                h1n = gp.tile([16, BL], F32R, tag="h1", bufs=2, name=f"h1_{t + 1}")
                gru_step(1, hh0[:, t + 1, :], h1[:], h1n[:])
                h1 = h1n

            if dbg:
                nc.sync.dma_start(dbg_xfm.ap(), x_fm[:])
                nc.sync.dma_start(dbg_h.ap(), hh0[:].bitcast(F32))

            psf = gps.tile([64, BL], F32, tag="g16", bufs=4, name="psf")
            nc.tensor.matmul(psf[:], S['clsT'][:], h1[:])
            nc.scalar.activation(gfeat[:], psf[:], AF.Identity, bias=S['clsb'][:])
            if dbg:
                nc.sync.dma_start(dbg_gf.ap(), gfeat[:].bitcast(F32))

            # ---------------- conv chunks ----------------
            with (
                tc.tile_pool(name="conv", bufs=1) as cp,
                tc.tile_pool(name="cpsum", bufs=1, space="PSUM") as cps,
            ):
                for c in range(NCH):
                    b0 = c * BCH
                    in2 = cp.tile([128, BCH, 8, 15], F32R, tag="in2", bufs=2)
                    nc.gpsimd.memset(in2[:, :, 0, :].bitcast(F32), 0.0)
                    nc.gpsimd.memset(in2[:, :, 7, :].bitcast(F32), 0.0)
                    nc.gpsimd.memset(in2[:, :, 1:7, 0:2].bitcast(F32), 0.0)
                    nc.gpsimd.memset(in2[:, :, 1:7, 14:15].bitcast(F32), 0.0)
                    for j in range(6):
                        band = cp.tile([72, BCH, 49], F32R, tag="band", bufs=2)
                        lo = 2 if j == 0 else 0
                        hi = 17 if j == 5 else 18
                        nc.gpsimd.memset(band[:, :, 0:2].bitcast(F32), 0.0)
                        nc.gpsimd.memset(band[:, :, 47:49].bitcast(F32), 0.0)
                        if j == 0:
                            nc.gpsimd.memset(band[0:8, :, 2:47].bitcast(F32), 0.0)
                        if j == 5:
                            # 32-aligned start; rows 64:68 re-filled by the DMA
                            nc.gpsimd.memset(band[64:72, :, 2:47].bitcast(F32), 0.0)
                        for ic in range(4):
                            dmae = nc.sync if ic % 2 == 0 else nc.scalar
                            dmae.dma_start(
                                band[4 * lo + ic:4 * (hi - 1) + ic + 1:4, :, 2:47],
                                y_d.ap()[b0:b0 + BCH, ic,
                                         6 * j - 2 + lo:6 * j - 2 + hi, :]
                                .rearrange("b h w -> h b w"))
                        ps = cps.tile([128, BCH, 12], F32, tag="cv", bufs=3,
                                      name="ps1")
                        for kx in range(15):
                            rhs = band[:, :, kx:kx + 34:3]
                            nc.tensor.matmul(
                                ps[:], w1sb[:, kx, :], rhs,
                                start=(kx == 0), stop=(kx == 14))
                        nc.scalar.activation(
                            in2[:, :, j + 1, 2:14], ps[:],
                            AF.Relu, bias=S['b1v'][:])

                    in3 = cp.tile([128, BCH, 9, 9], F32R, tag="in3", bufs=1)
                    nc.gpsimd.memset(in3[:, :, 0:2, :].bitcast(F32), 0.0)
                    nc.gpsimd.memset(in3[:, :, 8:9, :].bitcast(F32), 0.0)
                    nc.gpsimd.memset(in3[:, :, 2:8, 0:2].bitcast(F32), 0.0)
                    nc.gpsimd.memset(in3[:, :, 2:8, 8:9].bitcast(F32), 0.0)
                    for jp in range(3):
                        ps = cps.tile([128, BCH, 2, 6], F32, tag="cv", bufs=3,
                                      name="ps2")
                        for kyg in range(3):
                            for kx in range(5):
                                rhs = in2[:, :, 2 * jp + kyg:2 * jp + kyg + 2,
                                          kx:kx + 11:2]
                                k = kyg * 5 + kx
                                nc.tensor.matmul(
                                    ps[:], w2sb[:, k, :], rhs,
                                    start=(k == 0), stop=(k == 14))
                        nc.scalar.activation(
                            in3[:, :, 2 * jp + 2:2 * jp + 4, 2:8], ps[:],
                            AF.Relu, bias=S['b2v'][:])

                    for h, (fcin, b3n) in enumerate(((fcin0, 'b3a'), (fcin1, 'b3b'))):
                        ps = cps.tile([128, 3, 3, BCH], F32, tag="cv", bufs=3,
                                      name="ps3")
                        for ky in range(5):
                            for kx in range(5):
                                rhs = (in3[:, :, ky:ky + 5:2, kx:kx + 5:2]
                                       .rearrange("p b h w -> p h w b"))
                                k = ky * 5 + kx
                                nc.tensor.matmul(
                                    ps[:], w3sb[:, 25 * h + k, :], rhs,
                                    start=(k == 0), stop=(k == 24))
                        nc.scalar.activation(
                            fcin[:, b0:b0 + BCH, :]
                            .rearrange("p b (h w) -> p h w b", h=3),
                            ps[:], AF.Relu, bias=S[b3n][:])

            # ---------------- fc tail + heads + fusion ----------------
            with (
                tc.tile_pool(name="fcp", bufs=1) as fp,
                tc.tile_pool(name="fpsum", bufs=1, space="PSUM") as fps,
            ):
                fc1sb = fp.tile([128, 72, 128], DT.bfloat16, tag="fc1w")
                nc.sync.dma_start(fc1sb[:], pk['fc1p'].ap().rearrange("k p m -> p k m"))
                fc1a = fp.tile([128, 4, BL], F32R, tag="fc1a")
                for mt in range(4):
                    ps = fps.tile([128, BL], F32, tag="fc", bufs=1)
                    first = True
                    for h, fcin in enumerate((fcin0, fcin1)):
                        for p2 in range(9):
                            nc.tensor.matmul(
                                ps[:], fc1sb[:, h * 36 + p2 * 4 + mt, :],
                                fcin[:, :, p2],
                                start=first, stop=(h == 1 and p2 == 8))
                            first = False
                    nc.scalar.activation(fc1a[:, mt, :], ps[:], AF.Relu,
                                         bias=S['fc1b'][:, mt:mt + 1])

                psc = fps.tile([64, BL], F32, tag="fc", bufs=1, name="psc")
                for kc in range(4):
                    nc.tensor.matmul(psc[:], S['w23p'][:, kc * 64:(kc + 1) * 64],
                                     fc1a[:, kc, :],
                                     start=(kc == 0), stop=(kc == 3))
                cnnf = fp.tile([64, BL], F32R, tag="cnnf")
                nc.scalar.activation(cnnf[:], psc[:], AF.Identity, bias=S['b23v'][:])

                def head(feat, w1, b1, w2, b2, col):
                    ps1 = fps.tile([32, BL], F32, tag="hd", bufs=1, name="hps1")
                    nc.tensor.matmul(ps1[:], S[w1][:], feat[:])
                    hz = fp.tile([32, BL], F32R, tag="hz", bufs=2)
                    nc.scalar.activation(hz[:], ps1[:], AF.Relu, bias=S[b1][:])
                    ps2 = fps.tile([4, BL], F32, tag="hd", bufs=1, name="hps2")
                    nc.tensor.matmul(ps2[:], S[w2][:], hz[:])
                    ex = fp.tile([4, 2, BL], F32, tag="fus", bufs=5, name="ex")
                    nc.scalar.activation(ex[:, 0, :], ps2[:], AF.Exp, bias=S[b2][:])
                    nc.scalar.activation(e_sb[:, col, :], ex[:, 0, :], AF.Ln,
                                         bias=1.0)

                head(gfeat, 'wg1', 'bg1', 'wg2', 'bg2', 0)
                head(cnnf, 'wc1', 'bc1', 'wc2', 'bc2', 1)

                # fusion (all [4|1, 2, BL] tiles, partition base 0)
                psS = fps.tile([1, 2, BL], F32, tag="fu", bufs=1, name="psS")
                for hd in range(2):
                    nc.tensor.matmul(psS[:, hd, :], S['ones41'][:],
                                     e_sb[:, hd, :])
                s4 = fp.tile([1, 2, BL], F32, tag="fus", bufs=5, name="s4")
                nc.vector.tensor_scalar_add(s4[:], psS[:], 4.0)
                r2 = fp.tile([1, 2, BL], F32R, tag="fus", bufs=5, name="r2")
                with nc.allow_low_precision(reason="feeds f32r matmul (fp22)"):
                    nc.vector.reciprocal(r2[:], s4[:])
                psRB = fps.tile([4, 2, BL], F32, tag="fu", bufs=1, name="psRB")
                for hd in range(2):
                    nc.tensor.matmul(psRB[:, hd, :], S['bc14'][:],
                                     r2[:, hd, :])
                cc = fp.tile([4, 2, BL], F32, tag="fus", bufs=5, name="cc")
                nc.vector.tensor_mul(cc[:], e_sb[:], psRB[:])
                vb = fp.tile([4, 2, BL], F32, tag="fus", bufs=5, name="vb")
                nc.vector.tensor_scalar(vb[:], psRB[:], -4.0, 1.0,
                                        op0=ALU.mult, op1=ALU.add)
                t1 = fp.tile([4, 2, BL], F32, tag="fus", bufs=5, name="t1")
                nc.vector.tensor_mul(t1[:, 0, :], cc[:, 0, :], cc[:, 1, :])
                t2 = fp.tile([4, 2, BL], F32, tag="fus", bufs=5, name="t2")
                nc.vector.tensor_mul(t2[:], cc[:], vb[:])
                num = fp.tile([4, 2, BL], F32, tag="fus", bufs=5, name="num")
                nc.vector.tensor_add(num[:, 0, :], t1[:, 0, :], t2[:, 0, :])
                nc.vector.tensor_add(num[:, 1, :], num[:, 0, :], t2[:, 1, :])
                den = fp.tile([4, 2, BL], F32, tag="fus", bufs=5, name="den")
                nc.vector.tensor_mul(den[:, 0, :], vb[:, 0, :], vb[:, 1, :])
                nc.vector.reciprocal(den[:, 1, :], den[:, 0, :])
                evt = fp.tile([4, 2, BL], F32, tag="fus", bufs=5, name="evt")
                nc.vector.scalar_tensor_tensor(evt[:, 0, :], num[:, 1, :], 4.0,
                                               den[:, 1, :],
                                               op0=ALU.mult, op1=ALU.mult)
                at = fp.tile([4, 2, BL], F32, tag="fus", bufs=5, name="at")
                nc.vector.tensor_scalar_add(at[:], e_sb[:], 1.0)
                nc.sync.dma_start(ev_d.ap(), evt[:, 0, :])
                nc.sync.dma_start(a_d.ap(), at[:])

    nc.compile()
    return nc


_CACHE = {}


def kernel(x, y, params):
    x = np.ascontiguousarray(np.asarray(x, np.float32))
    y = np.ascontiguousarray(np.asarray(y, np.float32))
    packs = build_packs(params)
    if 'nc' not in _CACHE:
        _CACHE['nc'] = build_nc()
    nc = _CACHE['nc']

    import ml_dtypes
    shared = {n: np.ascontiguousarray(
        packs[n].astype(ml_dtypes.bfloat16) if n == 'fc1p' else packs[n])
        for n in PACK_SHAPES}
    in_maps = []
    for c in range(N_CORES):
        m = dict(shared)
        m['x'] = x[c * BL:(c + 1) * BL]
        m['y'] = y[c * BL:(c + 1) * BL]
        in_maps.append(m)
    res = run_bass_kernel_spmd(nc, in_maps, core_ids=list(range(N_CORES)))
    ev = np.concatenate([r['ev_t'].T for r in res.results], axis=0)
    ag = np.concatenate([r['a_t'][:, 0, :].T for r in res.results], axis=0)
    ac = np.concatenate([r['a_t'][:, 1, :].T for r in res.results], axis=0)
    return ev, ag, ac


# revision 29
# speedup vs baseline: 1.0700x; 1.0700x over previous
"""Trainium2 Bass kernel for nn_EvidentialLoNet (8-core data parallel).

Per core (B_local=512): CNN backbone (3 convs via f32r matmuls with
oh-pair column packing / ky-parity K-packing), 2-layer GRU (feature-major,
per-gate psums with matmul-accumulated input projections), evidential
heads + fusion. Everything on device except batch sharding and weight
packing.
"""
import numpy as np

import concourse.bacc as bacc
import concourse.mybir as mybir
from concourse import tile
from concourse.bass_utils import run_bass_kernel_spmd

AF = mybir.ActivationFunctionType
ALU = mybir.AluOpType
DT = mybir.dt
F32 = DT.float32
F32R = DT.float32r

N_CORES = 8
BL = 512          # batch per core
BCH = 32          # conv chunk batch
NCH = BL // BCH
EPS = 1e-5


# ------------------------------------------------------------------ packing

def _bn_fold(bp):
    s = np.asarray(bp['g'], np.float32) / np.sqrt(np.asarray(bp['v'], np.float32) + EPS)
    t = np.asarray(bp['b'], np.float32) - np.asarray(bp['m'], np.float32) * s
    return s, t


def build_packs(p):
    P = {k: np.asarray(v, np.float32) for k, v in p.items() if not isinstance(v, dict)}
    bn = {k: p[k] for k in ('bn1', 'bn2', 'bn3', 'bn_gru', 'hg_bn', 'hc_bn')}
    o = {}

    s1, t1 = _bn_fold(bn['bn1'])
    w1 = P['conv1_w'] * s1[:, None, None, None]
    b1 = P['conv1_b'] * s1 + t1
    s2, t2 = _bn_fold(bn['bn2'])
    w2 = P['conv2_w'] * s2[:, None, None, None]
    b2 = P['conv2_b'] * s2 + t2
    s3, t3 = _bn_fold(bn['bn3'])
    w3 = P['conv3_w'] * s3[:, None, None, None]
    b3 = P['conv3_b'] * s3 + t3

    w1p = np.zeros((15, 72, 128), np.float32)
    for kx in range(15):
        for ky in range(15):
            for ic in range(4):
                w1p[kx, ky * 4 + ic, 0:64] = w1[:, ic, ky, kx]
                w1p[kx, (ky + 3) * 4 + ic, 64:128] = w1[:, ic, ky, kx]
    o['w1p'] = w1p
    o['b1v'] = np.concatenate([b1, b1]).reshape(128, 1)

    w2p = np.zeros((15, 128, 128), np.float32)
    for kyg in range(3):
        for kx in range(5):
            for d in range(2):
                ky = 2 * kyg + d
                if ky < 5:
                    w2p[kyg * 5 + kx, d * 64:(d + 1) * 64, :] = w2[:, :, ky, kx].T
    o['w2p'] = w2p
    o['b2v'] = b2.reshape(128, 1)

    w3p = np.zeros((2, 25, 128, 128), np.float32)
    for h in range(2):
        for ky in range(5):
            for kx in range(5):
                w3p[h, ky * 5 + kx] = w3[h * 128:(h + 1) * 128, :, ky, kx].T
    o['w3p'] = w3p.reshape(50, 128, 128)
    o['b3a'] = b3[0:128].reshape(128, 1)
    o['b3b'] = b3[128:256].reshape(128, 1)

    fc1w, fc1b = P['fc1_w'], P['fc1_b']
    fc1p = np.zeros((2, 9, 4, 128, 128), np.float32)
    for h in range(2):
        for p2 in range(9):
            for mt in range(4):
                fc1p[h, p2, mt] = fc1w[mt * 128:(mt + 1) * 128,
                                       (h * 128 + np.arange(128)) * 9 + p2].T
    o['fc1p'] = fc1p.reshape(72, 128, 128)
    o['fc1b'] = fc1b.reshape(4, 128).T.copy()       # [128, 4] col per mtile

    W23 = P['fc3_w'] @ P['fc2_w']
    b23 = P['fc3_w'] @ P['fc2_b'] + P['fc3_b']
    w23p = np.zeros((128, 4, 64), np.float32)
    for kc in range(4):
        w23p[:, kc, :] = W23[:, kc * 128:(kc + 1) * 128].T
    o['w23p'] = w23p.reshape(128, 256)
    o['b23v'] = b23.reshape(64, 1)

    for nm, pre in (('hg', 'g'), ('hc', 'c')):
        s, t = _bn_fold(bn[nm + '_bn'])
        W1, B1 = P[nm + '_w1'], P[nm + '_b1']
        W2, B2 = P[nm + '_w2'], P[nm + '_b2']
        o[f'w{pre}1'] = W1.T.copy()
        o[f'b{pre}1'] = B1.reshape(32, 1)
        o[f'w{pre}2'] = (W2 * s[None, :]).T.copy()
        o[f'b{pre}2'] = (W2 @ t + B2).reshape(4, 1)

    for l in range(2):
        wih, whh = P[f'gru{l}_wih'], P[f'gru{l}_whh']
        bih, bhh = P[f'gru{l}_bih'], P[f'gru{l}_bhh']
        o[f'wih{l}T'] = wih.T.copy()                 # [I,48] cols r|z|n
        o[f'whh{l}T'] = whh.T.copy()                 # [16,48]
        o[f'br{l}'] = (bih[0:16] + bhh[0:16]).reshape(16, 1)
        o[f'bz{l}'] = (bih[16:32] + bhh[16:32]).reshape(16, 1)
        o[f'btn{l}'] = bih[32:48].reshape(16, 1)
        o[f'bhh{l}n'] = bhh[32:48].reshape(16, 1)
    sg, tg = _bn_fold(bn['bn_gru'])
    o['clsT'] = (P['cls_w'] * sg[None, :]).T.copy()  # [16,64]
    o['clsb'] = (P['cls_w'] @ tg + P['cls_b']).reshape(64, 1)

    o['ident'] = np.eye(128, dtype=np.float32)
    o['identf'] = np.eye(128, dtype=np.float32)
    o['ones41'] = np.ones((4, 1), np.float32)
    o['bc14'] = np.ones((1, 4), np.float32)
    return o


PACK_SHAPES = {
    'w1p': (15, 72, 128), 'b1v': (128, 1), 'w2p': (15, 128, 128), 'b2v': (128, 1),
    'w3p': (50, 128, 128), 'b3a': (128, 1), 'b3b': (128, 1),
    'fc1p': (72, 128, 128), 'fc1b': (128, 4), 'w23p': (128, 256), 'b23v': (64, 1),
    'wg1': (64, 32), 'bg1': (32, 1), 'wg2': (32, 4), 'bg2': (4, 1),
    'wc1': (64, 32), 'bc1': (32, 1), 'wc2': (32, 4), 'bc2': (4, 1),
    'wih0T': (4, 48), 'whh0T': (16, 48), 'br0': (16, 1), 'bz0': (16, 1),
    'btn0': (16, 1), 'bhh0n': (16, 1),
    'wih1T': (16, 48), 'whh1T': (16, 48), 'br1': (16, 1), 'bz1': (16, 1),
    'btn1': (16, 1), 'bhh1n': (16, 1),
    'clsT': (16, 64), 'clsb': (64, 1),
    'ident': (128, 128), 'identf': (128, 128), 'ones41': (4, 1), 'bc14': (1, 4),
}


# ------------------------------------------------------------------ builder

def build_nc(dbg=False):
    nc = bacc.Bacc("TRN2", target_bir_lowering=False, debug=False, num_devices=1)

    y_d = nc.dram_tensor("y", [BL, 4, 45, 45], F32R, kind="ExternalInput")
    x_d = nc.dram_tensor("x", [BL, 14, 4], F32, kind="ExternalInput")
    R_PACKS = {'w1p', 'w2p', 'w3p', 'w23p', 'wg1', 'wg2', 'wc1', 'wc2',
               'wih0T', 'wih1T', 'whh0T', 'whh1T', 'clsT', 'ones41', 'bc14',
               'ident'}

    def _pdt(n):
        if n == 'fc1p':
            return DT.bfloat16
        return F32R if n in R_PACKS else F32

    pk = {n: nc.dram_tensor(n, list(s), _pdt(n), kind="ExternalInput")
          for n, s in PACK_SHAPES.items()}
    ev_d = nc.dram_tensor("ev_t", [4, BL], F32, kind="ExternalOutput")
    if dbg:
        dbg_xfm = nc.dram_tensor("dbg_xfm", [4, 14, BL], DT.bfloat16, kind="ExternalOutput")
        dbg_h = nc.dram_tensor("dbg_h", [16, 15, BL], F32, kind="ExternalOutput")
        dbg_gf = nc.dram_tensor("dbg_gf", [64, BL], F32, kind="ExternalOutput")
        dbg_xq = nc.dram_tensor("dbg_xq", [56, 4, 128], DT.bfloat16, kind="ExternalOutput")
    a_d = nc.dram_tensor("a_t", [4, 2, BL], F32, kind="ExternalOutput")

    with tile.TileContext(nc) as tc:
        with (
            tc.tile_pool(name="wp", bufs=1) as wp,
            tc.tile_pool(name="main", bufs=1) as mp,
            tc.tile_pool(name="grut", bufs=2) as gp,
            tc.tile_pool(name="gpsum", bufs=1, space="PSUM") as gps,
        ):
            # ---------------- weights into SBUF ----------------
            w1sb = wp.tile([72, 15, 128], F32R, tag="w1")
            nc.sync.dma_start(w1sb[:], pk['w1p'].ap().rearrange("k p m -> p k m"))
            w2sb = wp.tile([128, 15, 128], F32R, tag="w2")
            nc.sync.dma_start(w2sb[:], pk['w2p'].ap().rearrange("k p m -> p k m"))
            w3sb = wp.tile([128, 50, 128], F32R, tag="w3")
            nc.sync.dma_start(w3sb[:], pk['w3p'].ap().rearrange("k p m -> p k m"))

            def _small(name):
                t = wp.tile(list(PACK_SHAPES[name]), _pdt(name), tag=name)
                nc.sync.dma_start(t[:], pk[name].ap())
                return t

            S = {n: _small(n) for n in PACK_SHAPES
                 if n not in ('w1p', 'w2p', 'w3p', 'fc1p')}
            ident = S['ident']

            # persistent activations
            fcin0 = mp.tile([128, BL, 9], DT.bfloat16, tag="fcin0")
            fcin1 = mp.tile([128, BL, 9], DT.bfloat16, tag="fcin1")
            e_sb = mp.tile([4, 2, BL], F32R, tag="e")
            gfeat = mp.tile([64, BL], F32R, tag="gfeat")
            hh0 = mp.tile([16, 15, BL], F32R, tag="hh0")    # L0 h history

            # ---------------- GRU prologue: x -> x_fm ----------------
            with (
                tc.tile_pool(name="prol", bufs=1) as pp,
                tc.tile_pool(name="tpp", bufs=2, space="PSUM") as tpp,
            ):
                xbm = pp.tile([128, 4, 56], F32, tag="xbm")
                nc.sync.dma_start(
                    xbm[:], x_d.ap().rearrange("(c p) t i -> p c (t i)", p=128))
                xq = pp.tile([56, 4, 128], DT.bfloat16, tag="xq")
                for bt in range(4):
                    tps = tpp.tile([56, 128], F32, tag="tp")
                    nc.tensor.transpose(tps[:], xbm[:, bt, :], S['identf'][:])
                    nc.vector.tensor_copy(xq[:, bt, :], tps[:])
                if dbg:
                    nc.sync.dma_start(dbg_xq.ap(), xq[:])
                x_fm = mp.tile([4, 14, BL], DT.bfloat16, tag="x_fm")
                for i in range(4):
                    nc.sync.dma_start(
                        x_fm[i:i + 1, :, :].rearrange("o t (c b) -> o t c b", c=4),
                        xq[i:i + 1 + 4 * 13:4, :, :])

            # ---------------- GRU ----------------
            wih0bf = mp.tile([4, 48], DT.bfloat16, tag="wih0bf")
            nc.vector.tensor_copy(wih0bf[:], S['wih0T'][:])
            nc.gpsimd.memset(hh0[:, 0, :].bitcast(F32), 0.0)
            h1_0 = gp.tile([16, BL], F32R, tag="h1", bufs=2)
            nc.gpsimd.memset(h1_0[:].bitcast(F32), 0.0)

            def gru_step(l, x_ap, h_prev, h_out):
                whh = S[f'whh{l}T']
                if l == 0:
                    xw = lambda a, b: wih0bf[:, a:b]
                    xr = x_ap
                else:
                    xw = lambda a, b: S['wih1T'][:, a:b]
                    xr = x_ap
                psr = gps.tile([16, BL], F32, tag="g16", bufs=2, name="psr")
                nc.tensor.matmul(psr[:], whh[:, 0:16], h_prev,
                                 start=True, stop=False)
                nc.tensor.matmul(psr[:], xw(0, 16), xr,
                                 start=False, stop=True)
                psz = gps.tile([16, BL], F32, tag="g16", bufs=2, name="psz")
                nc.tensor.matmul(psz[:], whh[:, 16:32], h_prev,
                                 start=True, stop=False)
                nc.tensor.matmul(psz[:], xw(16, 32), xr,
                                 start=False, stop=True)
                psn = gps.tile([16, BL], F32, tag="g16", bufs=2, name="psn")
                nc.tensor.matmul(psn[:], whh[:, 32:48], h_prev)
                r = gp.tile([16, BL], F32, tag="r", bufs=1)
                nc.scalar.activation(r[:], psr[:], AF.Sigmoid, bias=S[f'br{l}'][:])
                z = gp.tile([16, BL], F32, tag="z", bufs=1)
                nc.scalar.activation(z[:], psz[:], AF.Sigmoid, bias=S[f'bz{l}'][:])
                nm = gp.tile([16, BL], F32R, tag="nm", bufs=1)
                nc.vector.scalar_tensor_tensor(nm[:], psn[:], S[f'bhh{l}n'][:], r[:],
                                               op0=ALU.add, op1=ALU.mult)
                psna = gps.tile([16, BL], F32, tag="g16", bufs=2, name="psna")
                nc.tensor.matmul(psna[:], xw(32, 48), xr,
                                 start=True, stop=False)
                nc.tensor.matmul(psna[:], ident[0:16, 0:16], nm[:],
                                 start=False, stop=True)
                nt = gp.tile([16, BL], F32, tag="nt", bufs=1)
                nc.scalar.activation(nt[:], psna[:], AF.Tanh, bias=S[f'btn{l}'][:])
                d = gp.tile([16, BL], F32, tag="d", bufs=1)
                nc.vector.tensor_sub(d[:], h_prev, nt[:])
                zd = gp.tile([16, BL], F32, tag="zd", bufs=1)
                nc.vector.tensor_mul(zd[:], z[:], d[:])
                nc.vector.tensor_add(h_out, nt[:], zd[:])

            for t in range(14):
                gru_step(0, x_fm[:, t, :], hh0[:, t, :], hh0[:, t + 1, :])
            h1 = h1_0
            for t in range(14):
                h1n = gp.tile([16, BL], F32R, tag="h1", bufs=2, name=f"h1_{t + 1}")
                gru_step(1, hh0[:, t + 1, :], h1[:], h1n[:])
                h1 = h1n

            if dbg:
                nc.sync.dma_start(dbg_xfm.ap(), x_fm[:])
                nc.sync.dma_start(dbg_h.ap(), hh0[:].bitcast(F32))

            psf = gps.tile([64, BL], F32, tag="g16", bufs=2, name="psf")
            nc.tensor.matmul(psf[:], S['clsT'][:], h1[:])
            nc.scalar.activation(gfeat[:], psf[:], AF.Identity, bias=S['clsb'][:])
            if dbg:
                nc.sync.dma_start(dbg_gf.ap(), gfeat[:].bitcast(F32))

            # ---------------- conv chunks ----------------
            with (
                tc.tile_pool(name="conv", bufs=1) as cp,
                tc.tile_pool(name="cpsum", bufs=1, space="PSUM") as cps,
            ):
                for c in range(NCH):
                    b0 = c * BCH
                    in2 = cp.tile([128, BCH, 8, 15], F32R, tag="in2", bufs=2)
                    nc.gpsimd.memset(in2[:, :, 0, :].bitcast(F32), 0.0)
                    nc.gpsimd.memset(in2[:, :, 7, :].bitcast(F32), 0.0)
                    nc.gpsimd.memset(in2[:, :, 1:7, 0:2].bitcast(F32), 0.0)
                    nc.gpsimd.memset(in2[:, :, 1:7, 14:15].bitcast(F32), 0.0)
                    for j in range(6):
                        band = cp.tile([72, BCH, 49], F32R, tag="band", bufs=2)
                        lo = 2 if j == 0 else 0
                        hi = 17 if j == 5 else 18
                        nc.gpsimd.memset(band[:, :, 0:2].bitcast(F32), 0.0)
                        nc.gpsimd.memset(band[:, :, 47:49].bitcast(F32), 0.0)
                        if j == 0:
                            nc.gpsimd.memset(band[0:8, :, 2:47].bitcast(F32), 0.0)
                        if j == 5:
                            # 32-aligned start; rows 64:68 re-filled by the DMA
                            nc.gpsimd.memset(band[64:72, :, 2:47].bitcast(F32), 0.0)
                        for ic in range(4):
                            dmae = nc.sync if ic % 2 == 0 else nc.scalar
                            dmae.dma_start(
                                band[4 * lo + ic:4 * (hi - 1) + ic + 1:4, :, 2:47],
                                y_d.ap()[b0:b0 + BCH, ic,
                                         6 * j - 2 + lo:6 * j - 2 + hi, :]
                                .rearrange("b h w -> h b w"))
                        ps = cps.tile([128, BCH, 12], F32, tag="cv1", bufs=2,
                                      name="ps1")
                        for kx in range(15):
                            rhs = band[:, :, kx:kx + 34:3]
                            nc.tensor.matmul(
                                ps[:], w1sb[:, kx, :], rhs,
                                start=(kx == 0), stop=(kx == 14))
                        nc.scalar.activation(
                            in2[:, :, j + 1, 2:14], ps[:],
                            AF.Relu, bias=S['b1v'][:])

                    in3 = cp.tile([128, BCH, 9, 9], F32R, tag="in3", bufs=2)
                    nc.gpsimd.memset(in3[:, :, 0:2, :].bitcast(F32), 0.0)
                    nc.gpsimd.memset(in3[:, :, 8:9, :].bitcast(F32), 0.0)
                    nc.gpsimd.memset(in3[:, :, 2:8, 0:2].bitcast(F32), 0.0)
                    nc.gpsimd.memset(in3[:, :, 2:8, 8:9].bitcast(F32), 0.0)
                    for jp in range(3):
                        ps = cps.tile([128, BCH, 2, 6], F32, tag="cv2", bufs=2,
                                      name="ps2")
                        for kyg in range(3):
                            for kx in range(5):
                                rhs = in2[:, :, 2 * jp + kyg:2 * jp + kyg + 2,
                                          kx:kx + 11:2]
                                k = kyg * 5 + kx
                                nc.tensor.matmul(
                                    ps[:], w2sb[:, k, :], rhs,
                                    start=(k == 0), stop=(k == 14))
                        nc.scalar.activation(
                            in3[:, :, 2 * jp + 2:2 * jp + 4, 2:8], ps[:],
                            AF.Relu, bias=S['b2v'][:])

                    for h, (fcin, b3n) in enumerate(((fcin0, 'b3a'), (fcin1, 'b3b'))):
                        ps = cps.tile([128, 3, 3, BCH], F32, tag="cv3", bufs=2,
                                      name="ps3")
                        for ky in range(5):
                            for kx in range(5):
                                rhs = (in3[:, :, ky:ky + 5:2, kx:kx + 5:2]
                                       .rearrange("p b h w -> p h w b"))
                                k = ky * 5 + kx
                                nc.tensor.matmul(
                                    ps[:], w3sb[:, 25 * h + k, :], rhs,
                                    start=(k == 0), stop=(k == 24))
                        nc.scalar.activation(
                            fcin[:, b0:b0 + BCH, :]
                            .rearrange("p b (h w) -> p h w b", h=3),
                            ps[:], AF.Relu, bias=S[b3n][:])

            # ---------------- fc tail + heads + fusion ----------------
            with (
                tc.tile_pool(name="fcp", bufs=1) as fp,
                tc.tile_pool(name="fpsum", bufs=1, space="PSUM") as fps,
            ):
                fc1sb = fp.tile([128, 72, 128], DT.bfloat16, tag="fc1w")
                nc.sync.dma_start(fc1sb[:], pk['fc1p'].ap().rearrange("k p m -> p k m"))
                fc1a = fp.tile([128, 4, BL], F32R, tag="fc1a")
                for mt in range(4):
                    ps = fps.tile([128, BL], F32, tag="fc", bufs=1)
                    first = True
                    for h, fcin in enumerate((fcin0, fcin1)):
                        for p2 in range(9):
                            nc.tensor.matmul(
                                ps[:], fc1sb[:, h * 36 + p2 * 4 + mt, :],
                                fcin[:, :, p2],
                                start=first, stop=(h == 1 and p2 == 8))
                            first = False
                    nc.scalar.activation(fc1a[:, mt, :], ps[:], AF.Relu,
                                         bias=S['fc1b'][:, mt:mt + 1])

                psc = fps.tile([64, BL], F32, tag="fc", bufs=1, name="psc")
                for kc in range(4):
                    nc.tensor.matmul(psc[:], S['w23p'][:, kc * 64:(kc + 1) * 64],
                                     fc1a[:, kc, :],
                                     start=(kc == 0), stop=(kc == 3))
                cnnf = fp.tile([64, BL], F32R, tag="cnnf")
                nc.scalar.activation(cnnf[:], psc[:], AF.Identity, bias=S['b23v'][:])

                def head(feat, w1, b1, w2, b2, col):
                    ps1 = fps.tile([32, BL], F32, tag="hd", bufs=1, name="hps1")
                    nc.tensor.matmul(ps1[:], S[w1][:], feat[:])
                    hz = fp.tile([32, BL], F32R, tag="hz", bufs=2)
                    nc.scalar.activation(hz[:], ps1[:], AF.Relu, bias=S[b1][:])
                    ps2 = fps.tile([4, BL], F32, tag="hd", bufs=1, name="hps2")
                    nc.tensor.matmul(ps2[:], S[w2][:], hz[:])
                    ex = fp.tile([4, 2, BL], F32, tag="fus", bufs=5, name="ex")
                    nc.scalar.activation(ex[:, 0, :], ps2[:], AF.Exp, bias=S[b2][:])
                    nc.scalar.activation(e_sb[:, col, :], ex[:, 0, :], AF.Ln,
                                         bias=1.0)

                head(gfeat, 'wg1', 'bg1', 'wg2', 'bg2', 0)
                head(cnnf, 'wc1', 'bc1', 'wc2', 'bc2', 1)

                # fusion (all [4|1, 2, BL] tiles, partition base 0)
                psS = fps.tile([1, 2, BL], F32, tag="fu", bufs=1, name="psS")
                for hd in range(2):
                    nc.tensor.matmul(psS[:, hd, :], S['ones41'][:],
                                     e_sb[:, hd, :])
                s4 = fp.tile([1, 2, BL], F32, tag="fus", bufs=5, name="s4")
                nc.vector.tensor_scalar_add(s4[:], psS[:], 4.0)
                r2 = fp.tile([1, 2, BL], F32R, tag="fus", bufs=5, name="r2")
                with nc.allow_low_precision(reason="feeds f32r matmul (fp22)"):
                    nc.vector.reciprocal(r2[:], s4[:])
                psRB = fps.tile([4, 2, BL], F32, tag="fu", bufs=1, name="psRB")
                for hd in range(2):
                    nc.tensor.matmul(psRB[:, hd, :], S['bc14'][:],
                                     r2[:, hd, :])
                cc = fp.tile([4, 2, BL], F32, tag="fus", bufs=5, name="cc")
                nc.vector.tensor_mul(cc[:], e_sb[:], psRB[:])
                vb = fp.tile([4, 2, BL], F32, tag="fus", bufs=5, name="vb")
                nc.vector.tensor_scalar(vb[:], psRB[:], -4.0, 1.0,
                                        op0=ALU.mult, op1=ALU.add)
                t1 = fp.tile([4, 2, BL], F32, tag="fus", bufs=5, name="t1")
                nc.vector.tensor_mul(t1[:, 0, :], cc[:, 0, :], cc[:, 1, :])
                t2 = fp.tile([4, 2, BL], F32, tag="fus", bufs=5, name="t2")
                nc.vector.tensor_mul(t2[:], cc[:], vb[:])
                num = fp.tile([4, 2, BL], F32, tag="fus", bufs=5, name="num")
                nc.vector.tensor_add(num[:, 0, :], t1[:, 0, :], t2[:, 0, :])
                nc.vector.tensor_add(num[:, 1, :], num[:, 0, :], t2[:, 1, :])
                den = fp.tile([4, 2, BL], F32, tag="fus", bufs=5, name="den")
                nc.vector.tensor_mul(den[:, 0, :], vb[:, 0, :], vb[:, 1, :])
                nc.vector.reciprocal(den[:, 1, :], den[:, 0, :])
                evt = fp.tile([4, 2, BL], F32, tag="fus", bufs=5, name="evt")
                nc.vector.scalar_tensor_tensor(evt[:, 0, :], num[:, 1, :], 4.0,
                                               den[:, 1, :],
                                               op0=ALU.mult, op1=ALU.mult)
                at = fp.tile([4, 2, BL], F32, tag="fus", bufs=5, name="at")
                nc.vector.tensor_scalar_add(at[:], e_sb[:], 1.0)
                nc.sync.dma_start(ev_d.ap(), evt[:, 0, :])
                nc.sync.dma_start(a_d.ap(), at[:])

    nc.compile()
    return nc


_CACHE = {}


def kernel(x, y, params):
    x = np.ascontiguousarray(np.asarray(x, np.float32))
    y = np.ascontiguousarray(np.asarray(y, np.float32))
    packs = build_packs(params)
    if 'nc' not in _CACHE:
        _CACHE['nc'] = build_nc()
    nc = _CACHE['nc']

    import ml_dtypes
    shared = {n: np.ascontiguousarray(
        packs[n].astype(ml_dtypes.bfloat16) if n == 'fc1p' else packs[n])
        for n in PACK_SHAPES}
    in_maps = []
    for c in range(N_CORES):
        m = dict(shared)
        m['x'] = x[c * BL:(c + 1) * BL]
        m['y'] = y[c * BL:(c + 1) * BL]
        in_maps.append(m)
    res = run_bass_kernel_spmd(nc, in_maps, core_ids=list(range(N_CORES)))
    ev = np.concatenate([r['ev_t'].T for r in res.results], axis=0)
    ag = np.concatenate([r['a_t'][:, 0, :].T for r in res.results], axis=0)
    ac = np.concatenate([r['a_t'][:, 1, :].T for r in res.results], axis=0)
    return ev, ag, ac
